# revision 24
# baseline (speedup 1.0000x reference)
"""GAT (4-layer graph attention network) on 8 Trainium2 NeuronCores.

Design (v2, dst-per-partition slot layout):
  - Nodes sharded into 8 contiguous ranges; per core, local dst nodes are
    sorted by (lo-degree, hi-degree) and grouped into 128-node tiles.
  - Edges of a tile are laid out slot-major: V[d, j, :] = table row of the
    j-th source of dst d, fetched with one bulk gather per (tile, half)
    (table halves because gather indices are int16).  Pad slots gather row 0
    and are killed with a -100 score offset (exp -> ~0).
  - Per-node DRAM table rows hold [features*W' | s_src] where W' has the
    next layer's BN scale folded in; s_dst of local dsts is kept in SBUF as
    a per-partition column, added via the scalar engine's fused
    Lrelu(x + bias) -> Exp(accum_out=denominator).
  - Aggregation = DVE free-dim reduce of p-weighted gathered rows; no
    one-hot matmuls, no per-edge dst-score gather.
  - Layer 1's table (xW1, 260 cols in 384-elem rows) is built densely and
    replicated on every core; layers 2-4 build local rows and AllGather.
  - Final: per-graph mean-pool via one-hot batch matmul, AllReduce,
    replicated f32 MLP head.

kernel(**inputs) takes FULL inputs, returns the full [B, C] f32 output.
"""

import math
from contextlib import ExitStack

import numpy as np
import ml_dtypes

N_CORES = 8
NEG = 0.2
EPS = 1e-5
P = 128
DEF_SL = 2048      # dense-phase xT streaming slab columns
CHUNK_CAP = 8      # max 128-idx chunks per gather call (SWDGE limit)
MNEG = -100.0
SCALAR_COPY = True
EDGE_LEVEL = 9  # 0=gathers,1=+scores,2=+exp,3=+reduce/U,4=+finalize,9=full      # score offset for pad slots: lrelu -> -20, exp -> ~2e-9

BF = ml_dtypes.bfloat16


def cdiv(a, b):
    return -(-a // b)


# ----------------------------------------------------------------------------
# Host-side planning / preprocessing
# ----------------------------------------------------------------------------

class Plan:
    """Static program structure (cross-core maxima so SPMD program is shared).

    Per core: local dsts sorted by (lo_deg, hi_deg) desc, tiles of 128.
    m_lo[t]/m_hi[t] = max over cores of per-tile max per-dst lo/hi degree.
    """

    def __init__(self, N, E, B, IN, HID, Hh, n_cores, edge_index):
        self.N, self.E, self.B, self.IN, self.HID, self.Hh = N, E, B, IN, HID, Hh
        self.n_cores = n_cores
        self.npc = N // n_cores
        self.T = cdiv(self.npc, P)
        self.half = (n_cores // 2) * self.npc          # 25000: cores 0-3 = lo
        src = np.asarray(edge_index[0], np.int64)
        dst = np.asarray(edge_index[1], np.int64)

        npc, T = self.npc, self.T
        # Node->core assignment, two levels:
        #  1) half assignment (cores 0..3 = table rows < half) by a greedy
        #     discrepancy pass so every dst's sources split ~evenly between
        #     halves (shrinks the per-tile max-lo + max-hi slot padding);
        #  2) within each half, stride nodes over the 4 cores in total-degree
        #     order so all cores see near-identical degree profiles (keeps
        #     the shared SPMD cross-core maxima tight).
        deg = np.bincount(dst, minlength=N)
        out_deg = np.bincount(src, minlength=N)
        gorder = np.argsort(-out_deg, kind="stable")
        eo_s = np.argsort(src, kind="stable")
        dst_by_src = dst[eo_s]
        sstarts = np.searchsorted(src[eo_s], np.arange(N + 1))
        imb = np.zeros(N, np.int32)
        cap_lo = N // 2
        n_lo = 0
        n_hi = 0
        is_lo = np.zeros(N, bool)
        for n in gorder:
            ds = dst_by_src[sstarts[n]:sstarts[n + 1]]
            if (imb[ds].sum() < 0 and n_lo < cap_lo) or n_hi >= N - cap_lo:
                is_lo[n] = True
                n_lo += 1
                imb[ds] += 1
            else:
                n_hi += 1
                imb[ds] -= 1
        half_cores = n_cores // 2
        lo_nodes = np.where(is_lo)[0]
        hi_nodes = np.where(~is_lo)[0]
        lo_sorted = lo_nodes[np.argsort(-deg[lo_nodes], kind="stable")]
        hi_sorted = hi_nodes[np.argsort(-deg[hi_nodes], kind="stable")]
        self.node_of = [lo_sorted[c::half_cores] for c in range(half_cores)] \
            + [hi_sorted[c::half_cores] for c in range(half_cores)]
        core_of = np.empty(N, np.int64)
        for c in range(n_cores):
            core_of[self.node_of[c]] = c

        self.adj = []             # per core: list over tiles of (lo_lists, hi_lists)
        m_lo = np.zeros((n_cores, T), np.int64)
        m_hi = np.zeros((n_cores, T), np.int64)
        src_lo = is_lo[src]
        for c in range(n_cores):
            mine = core_of[dst] == c
            s_c = src[mine]
            slo_c = src_lo[mine]
            # local dst index within this core's preliminary order
            lidx = np.empty(N, np.int64)
            lidx[self.node_of[c]] = np.arange(npc)
            d_c = lidx[dst[mine]]
            lo_cnt = np.bincount(d_c[slo_c], minlength=npc)
            hi_cnt = np.bincount(d_c[~slo_c], minlength=npc)
            perm = np.lexsort((-hi_cnt, -lo_cnt))
            self.node_of[c] = self.node_of[c][perm]     # final tile order
            eo = np.argsort(d_c, kind="stable")
            s_sorted = s_c[eo]
            slo_sorted = slo_c[eo]
            starts = np.searchsorted(d_c[eo], np.arange(npc + 1))
            tiles = []
            for t in range(T):
                lo_lists, hi_lists = [], []
                for d in perm[t * P: (t + 1) * P]:
                    e = s_sorted[starts[d]:starts[d + 1]]
                    sl = slo_sorted[starts[d]:starts[d + 1]]
                    lo_lists.append(e[sl])
                    hi_lists.append(e[~sl])
                if len(lo_lists) < P:       # pad partial tile
                    pad = P - len(lo_lists)
                    lo_lists += [np.empty(0, np.int64)] * pad
                    hi_lists += [np.empty(0, np.int64)] * pad
                tiles.append((lo_lists, hi_lists))
                m_lo[c, t] = max(len(x) for x in lo_lists)
                m_hi[c, t] = max(len(x) for x in hi_lists)
            self.adj.append(tiles)
        self.m_lo = np.maximum(m_lo.max(axis=0), 1).astype(np.int64)
        self.m_hi = m_hi.max(axis=0).astype(np.int64)
        self.m_tot = self.m_lo + self.m_hi
        # column offsets into mneg table [P, sum(m_tot)]
        self.moff = np.concatenate([[0], np.cumsum(self.m_tot)]).astype(np.int64)
        self.MCOLS = int(self.moff[-1])
        # gidx column layout: per tile: 8*m_lo cols then 8*m_hi cols
        self.g_lo_off = []
        self.g_hi_off = []
        go = 0
        for t in range(T):
            self.g_lo_off.append(go)
            go += 8 * int(self.m_lo[t])
            self.g_hi_off.append(go)
            go += 8 * int(self.m_hi[t])
        self.GCOLS = go
        # global permuted position of each node (gather index base):
        # node_of[c][i] sits at table row c*npc + i
        gpos = np.empty(N, np.int64)
        for c in range(n_cores):
            gpos[self.node_of[c]] = c * npc + np.arange(npc)
        self.gpos = gpos


def _wrap16(vals16):
    """[n] -> [128, n/16] int16: 16-partition-wrapped, replicated x8."""
    n = vals16.shape[0]
    assert n % 16 == 0
    a = vals16.reshape(n // 16, 16).T.astype(np.int16)
    return np.tile(a, (8, 1))


def preprocess(inputs, n_cores=N_CORES):
    x = np.asarray(inputs["x"], np.float32)
    edge_index = np.asarray(inputs["edge_index"])
    batch = np.asarray(inputs["batch"], np.int64)
    N, IN = x.shape
    E = edge_index.shape[1]
    a_src1 = np.asarray(inputs["a_src1"], np.float32)
    Hh, HID = a_src1.shape
    C = np.asarray(inputs["Wh2"], np.float32).shape[1]
    B = 64 if N == 50000 else int(batch.max()) + 1

    plan = Plan(N, E, B, IN, HID, Hh, n_cores, edge_index)
    npc, T, half = plan.npc, plan.T, plan.half

    HF = Hh * HID                 # 256
    R1 = 512                      # layer-1 fp8 row elems (256 fp8 + 4 bf16 scores)
    R2 = 128                      # layer 2-4 table row elems (65 used)
    gs = 1.0 / math.sqrt(1.0 + EPS)

    def fold(W, a_s, a_d, g=None):
        """[W*diag(g)*gs | W@a_s per head | W@a_d per head]"""
        W = np.asarray(W, np.float32)
        a_s = np.asarray(a_s, np.float32)
        a_d = np.asarray(a_d, np.float32)
        Fin = W.shape[0]
        hh, F = a_s.shape
        Wr = W.reshape(Fin, hh, F)
        ws = np.einsum("ihf,hf->ih", Wr, a_s)
        wd = np.einsum("ihf,hf->ih", Wr, a_d)
        Wsc = W if g is None else W * (np.asarray(g, np.float32) * gs)[None, :]
        return np.concatenate([Wsc, ws, wd], axis=1).astype(BF)

    w1p = fold(inputs["W1"], a_src1, inputs["a_dst1"])                # [128,264]
    w2p = fold(inputs["W2"], inputs["a_src2"], inputs["a_dst2"],
               inputs["g2"])                                          # [256,66]
    nq2 = HF // P
    w2p = np.concatenate([w2p[q * P:(q + 1) * P, :] for q in range(nq2)],
                         axis=1)                                      # [128,132]
    w3p = fold(inputs["W3"], inputs["a_src3"], inputs["a_dst3"], inputs["g3"])
    w4p = fold(inputs["W4"], inputs["a_src4"], inputs["a_dst4"], inputs["g4"])

    # bias vectors after BN fold: y = U'*rc + bbv ; scalar path iff bbv == 0
    def bbv(g, b, be):
        return (np.asarray(g, np.float32) * gs * np.asarray(b, np.float32)
                + np.asarray(be, np.float32))

    b1 = np.asarray(inputs["b1"], np.float32)
    bb = [b1, bbv(inputs["g2"], inputs["b2"], inputs["be2"]),
          bbv(inputs["g3"], inputs["b3"], inputs["be3"]),
          bbv(inputs["g4"], inputs["b4"], inputs["be4"])]
    triv = [bool(np.all(np.abs(v) < 1e-30)) for v in bb]
    bbrep = [np.tile(v[None, :], (P, 1)).astype(np.float32) for v in bb]

    wh1 = np.asarray(inputs["Wh1"], np.float32)
    MH = wh1.shape[1]
    bh1rep = np.tile(np.asarray(inputs["bh1"], np.float32)[None, :], (B, 1))
    wh2 = np.asarray(inputs["Wh2"], np.float32)
    bh2rep = np.tile(np.asarray(inputs["bh2"], np.float32)[None, :], (B, 1))

    # globally permuted x^T for the dense phase (every core builds full table1)
    permg = np.concatenate([plan.node_of[c] for c in range(n_cores)])
    xTg = np.ascontiguousarray(x[permg].T).astype(BF)                 # [IN, N]
    idbf = np.eye(P, dtype=np.float32).astype(BF)
    idf32 = np.eye(P, dtype=np.float32)
    iota = np.tile(np.arange(P, dtype=np.float32)[None, :], (P, 1)).astype(BF)
    onescol = np.ones((P, 1), np.float32).astype(BF)

    common = dict(xTg=xTg, w1p=w1p, w2p=w2p, w3p=w3p, w4p=w4p,
                  bb1=bbrep[0][:, :HF], bb2=bbrep[1], bb3=bbrep[2],
                  bb4=bbrep[3], wh1=wh1, bh1rep=bh1rep, wh2=wh2,
                  bh2rep=bh2rep, idbf=idbf, idf32=idf32, iota=iota,
                  onescol=onescol)

    per_core = []
    for c in range(n_cores):
        gidx = np.zeros((P, max(plan.GCOLS, 8)), np.int16)
        mneg = np.full((P, max(plan.MCOLS, 1)), MNEG, np.float32)
        for t in range(T):
            lo_lists, hi_lists = plan.adj[c][t]
            for which, lists, m, goff in (
                    (0, lo_lists, int(plan.m_lo[t]), plan.g_lo_off[t]),
                    (1, hi_lists, int(plan.m_hi[t]), plan.g_hi_off[t])):
                if m == 0:
                    continue
                iv = np.zeros((P, m), np.int16)       # [dst, slot]
                for d, e in enumerate(lists):
                    ge = plan.gpos[e] - (half if which else 0)
                    iv[d, :len(e)] = ge.astype(np.int16)
                    if which == 0:
                        mneg[d, plan.moff[t]:plan.moff[t] + len(e)] = 0.0
                    else:
                        o = plan.moff[t] + int(plan.m_lo[t])
                        mneg[d, o:o + len(e)] = 0.0
                # slot-major: position j*128+d  -> value iv[d, j]
                vals = iv.T.reshape(-1)               # [m*128]
                gidx[:, goff:goff + 8 * m] = _wrap16(vals)

        batchv = np.full((P, T), -1.0, np.float32)
        bperm = batch[plan.node_of[c]].astype(np.float32)
        for t in range(T):
            nt = min((t + 1) * P, npc) - t * P
            batchv[:nt, t] = bperm[t * P:t * P + nt]

        xtl = np.zeros((IN, T * P), BF)
        xtl[:, :npc] = x[plan.node_of[c]].T.astype(BF)
        per_core.append(dict(gidx=gidx, mneg=mneg.astype(BF),
                             batchv=batchv.astype(BF), xtl=xtl))

    meta = dict(plan=plan, HF=HF, R1=R1, R2=R2, C=C, MH=MH, B=B, triv=triv)
    return meta, common, per_core


# ----------------------------------------------------------------------------
# Bass program (shared by all cores; per-core behavior differs only via data)
# ----------------------------------------------------------------------------

def build_program(meta, debug_dumps=False):
    import concourse.bass as bass
    import concourse.bacc as bacc
    import concourse.mybir as mybir
    import concourse.tile as tile

    F32 = mybir.dt.float32
    BF16 = mybir.dt.bfloat16
    FP8 = mybir.dt.float8e4
    I16 = mybir.dt.int16
    A = mybir.AluOpType
    ACT = mybir.ActivationFunctionType
    X = mybir.AxisListType.X

    plan = meta["plan"]
    N, IN, Hh, HID = plan.N, plan.IN, plan.Hh, plan.HID
    B, C, MH = meta["B"], meta["C"], meta["MH"]
    HF, R1, R2 = meta["HF"], meta["R1"], meta["R2"]
    triv = meta["triv"]
    npc, T, half = plan.npc, plan.T, plan.half
    n_cores = plan.n_cores
    SL = min(DEF_SL, N)

    nc = bacc.Bacc("TRN2", num_devices=n_cores, num_swdge_queues=4)
    rg = [list(range(n_cores))]

    def ein(name, shape, dt):
        return nc.dram_tensor(name, shape, dt, kind="ExternalInput")

    xTg_d = ein("xTg", [IN, N], BF16)
    xtl_d = ein("xtl", [IN, T * P], BF16)
    w1p_d = ein("w1p", [IN, HF + 2 * Hh], BF16)
    w2p_d = ein("w2p", [P, (HF // P) * (HID + 2)], BF16)
    w3p_d = ein("w3p", [HID, HID + 2], BF16)
    w4p_d = ein("w4p", [HID, HID + 2], BF16)
    bb1_d = ein("bb1", [P, HF], F32)
    bb_d = [None, ein("bb2", [P, HID], F32), ein("bb3", [P, HID], F32),
            ein("bb4", [P, HID], F32)]
    wh1_d = ein("wh1", [HID, MH], F32)
    bh1rep_d = ein("bh1rep", [B, MH], F32)
    wh2_d = ein("wh2", [MH, C], F32)
    bh2rep_d = ein("bh2rep", [B, C], F32)
    idbf_d = ein("idbf", [P, P], BF16)
    idf32_d = ein("idf32", [P, P], F32)
    iota_d = ein("iota", [P, P], BF16)
    ones_d = ein("onescol", [P, 1], BF16)
    gidx_d = ein("gidx", [P, max(plan.GCOLS, 8)], I16)
    mneg_d = ein("mneg", [P, max(plan.MCOLS, 1)], BF16)
    batchv_d = ein("batchv", [P, T], BF16)

    shr = "Shared" if n_cores > 4 else "Local"
    table1 = nc.dram_tensor("table1", [N, R1], FP8)
    tloc = [None, nc.dram_tensor("tloc2", [npc, R2], BF16),
            nc.dram_tensor("tloc3", [npc, R2], BF16),
            nc.dram_tensor("tloc4", [npc, R2], BF16)]
    tfull = [None,
             nc.dram_tensor("tfull2", [N, R2], BF16, addr_space=shr),
             nc.dram_tensor("tfull3", [N, R2], BF16, addr_space=shr),
             nc.dram_tensor("tfull4", [N, R2], BF16, addr_space=shr)]
    arin = nc.dram_tensor("arin", [HID, B + 1], F32)
    arout = nc.dram_tensor("arout", [HID, B + 1], F32, addr_space=shr)
    out_d = nc.dram_tensor("out", [B, C], F32, kind="ExternalOutput")
    dbg = {}
    if debug_dumps:
        dbg["x1"] = nc.dram_tensor("dbg_x1", [P, HF], F32, kind="ExternalOutput")
        dbg["den1"] = nc.dram_tensor("dbg_den1", [P, Hh], F32, kind="ExternalOutput")
        dbg["h2"] = nc.dram_tensor("dbg_h2", [P, HID], F32, kind="ExternalOutput")
        M0 = int(plan.m_tot[0])
        dbg["e0"] = nc.dram_tensor("dbg_e0", [P, M0 * Hh], F32, kind="ExternalOutput")
        dbg["U0"] = nc.dram_tensor("dbg_U0", [P, HF], F32, kind="ExternalOutput")
        dbg["rc0"] = nc.dram_tensor("dbg_rc0", [P, Hh], F32, kind="ExternalOutput")
        dbg["p0"] = nc.dram_tensor("dbg_p0", [P, M0 * Hh], F32, kind="ExternalOutput")
        dbg["h4"] = nc.dram_tensor("dbg_h4", [P, HID], F32, kind="ExternalOutput")

    gcnt = nc.gpsimd.alloc_register("gcnt")
    qctr = [0]

    def gather_split(out3, tab_ap, col0, n_chunks, elem, gidx_s):
        done = 0
        while done < n_chunks:
            nn = min(CHUNK_CAP, n_chunks - done)
            nc.gpsimd.reg_mov(gcnt, nn * P)
            nc.gpsimd.dma_gather(
                out3[:, done:done + nn, :], tab_ap,
                gidx_s[:, col0 + 8 * done: col0 + 8 * (done + nn)],
                nn * P, gcnt, elem, queue_num=qctr[0] % 4)
            qctr[0] += 1
            done += nn

    with ExitStack() as ctx:
        tc = ctx.enter_context(tile.TileContext(nc))
        cst = ctx.enter_context(tc.tile_pool(name="cst", bufs=1))
        vpool = ctx.enter_context(tc.tile_pool(name="vpool", bufs=2))
        v2pool = ctx.enter_context(tc.tile_pool(name="v2pool", bufs=4))
        wpool = ctx.enter_context(tc.tile_pool(name="wpool", bufs=1))
        fpool = ctx.enter_context(tc.tile_pool(name="fpool", bufs=2))
        spool = ctx.enter_context(tc.tile_pool(name="spool", bufs=2))
        xpool = ctx.enter_context(tc.tile_pool(name="xpool", bufs=2))
        hpool = ctx.enter_context(tc.tile_pool(name="hpool", bufs=1))
        tpool = ctx.enter_context(tc.tile_pool(name="tpool", bufs=2, space="PSUM"))

        def load_const(dram, shape, dt, name):
            t = cst.tile(shape, dt, name=name, tag=name)
            nc.sync.dma_start(out=t[:], in_=dram[:])
            return t

        w1p_s = load_const(w1p_d, [IN, HF + 2 * Hh], BF16, "w1p_s")
        w2p_s = load_const(w2p_d, [P, (HF // P) * (HID + 2)], BF16, "w2p_s")
        w3p_s = load_const(w3p_d, [HID, HID + 2], BF16, "w3p_s")
        w4p_s = load_const(w4p_d, [HID, HID + 2], BF16, "w4p_s")
        wlp_s = [None, w2p_s, w3p_s, w4p_s]
        bb1_s = load_const(bb1_d, [P, HF], F32, "bb1_s")
        bb_s = [None] + [load_const(bb_d[i], [P, HID], F32, f"bb{i+1}_s")
                         for i in (1, 2, 3)]
        wh1_s = load_const(wh1_d, [HID, MH], F32, "wh1_s")
        bh1rep_s = load_const(bh1rep_d, [B, MH], F32, "bh1rep_s")
        wh2_s = load_const(wh2_d, [MH, C], F32, "wh2_s")
        bh2rep_s = load_const(bh2rep_d, [B, C], F32, "bh2rep_s")
        idbf_s = load_const(idbf_d, [P, P], BF16, "idbf_s")
        idf32_s = load_const(idf32_d, [P, P], F32, "idf32_s")
        iota_s = load_const(iota_d, [P, P], BF16, "iota_s")
        ones_s = load_const(ones_d, [P, 1], BF16, "ones_s")
        gidx_s = load_const(gidx_d, [P, max(plan.GCOLS, 8)], I16, "gidx_s")
        mneg_s = load_const(mneg_d, [P, max(plan.MCOLS, 1)], BF16, "mneg_s")
        batchv_s = load_const(batchv_d, [P, T], BF16, "batchv_s")
        xtl_s = load_const(xtl_d, [IN, T * P], BF16, "xtl_s")

        # persistent per-tile state
        sdst = {1: [], 2: [], 3: [], 4: []}   # [P, Hl] f32 per tile (layer l)
        h_keep = {2: [], 3: []}
        for t in range(T):
            sdst[1].append(hpool.tile([P, Hh], F32, tag=f"sd1_{t}",
                                      name=f"sd1_{t}"))
            for l in (2, 3, 4):
                sdst[l].append(hpool.tile([P, 1], F32, tag=f"sd{l}_{t}",
                                          name=f"sd{l}_{t}"))
            h_keep[2].append(hpool.tile([P, HID], BF16, tag=f"h2_{t}",
                                        name=f"h2_{t}"))
            h_keep[3].append(hpool.tile([P, HID], BF16, tag=f"h3_{t}",
                                        name=f"h3_{t}"))

        psA, _freeA = tc.tile([HID, B], F32, space="PSUM", name="psA")
        psB, _freeB = tc.tile([B, 1], F32, space="PSUM", name="psB")

        # ---------------- dense phase: table1 rows (replicated, permuted) ---
        for sb in range(cdiv(N, SL)):
            c0 = sb * SL
            c1 = min(c0 + SL, N)
            xsl = xpool.tile([IN, c1 - c0], BF16, tag="xsl", name=f"xsl{sb}")
            nc.sync.dma_start(out=xsl[:], in_=xTg_d[:, c0:c1])
            for blk in range(c0 // P, cdiv(c1, P)):
                b0 = blk * P
                b1_ = min(b0 + P, N)
                nb = b1_ - b0
                ps = tpool.tile([P, HF + 2 * Hh], F32, tag="tN",
                                name=f"psd{blk}")
                nc.tensor.matmul(ps[:nb, :], lhsT=xsl[:, b0 - c0:b1_ - c0],
                                 rhs=w1p_s[:], start=True, stop=True)
                tb = spool.tile([P, HF + 2 * Hh], FP8, tag="tbd",
                                name=f"tbd{blk}")
                if blk % 2 == 0 or not SCALAR_COPY:
                    nc.vector.tensor_copy(tb[:nb, 0:HF], ps[:nb, 0:HF])
                else:
                    nc.scalar.activation(out=tb[:nb, 0:HF], in_=ps[:nb, 0:HF],
                                         func=ACT.Identity)
                nc.vector.tensor_copy(
                    tb[:nb, HF:HF + 2 * Hh].bitcast(BF16),
                    ps[:nb, HF:HF + Hh])
                nc.sync.dma_start(out=table1[b0:b1_, 0:HF + 2 * Hh],
                                  in_=tb[:nb, :])
        # local s_dst for layer 1 (from zero-padded local xT)
        for t in range(T):
            psd = tpool.tile([P, Hh], F32, tag="tN", name=f"psd2_{t}")
            nc.tensor.matmul(psd[:], lhsT=xtl_s[:, t * P:(t + 1) * P],
                             rhs=w1p_s[:, HF + Hh:HF + 2 * Hh],
                             start=True, stop=True)
            if SCALAR_COPY:
                nc.scalar.activation(out=sdst[1][t][:], in_=psd[:],
                                     func=ACT.Identity)
            else:
                nc.vector.tensor_copy(sdst[1][t][:], psd[:])

        # ---------------- edge phase ----------------
        def edge_phase(l):
            R = R1 if l == 1 else R2
            F = HF if l == 1 else HID
            Hl = Hh if l == 1 else 1
            tab = table1 if l == 1 else tfull[l - 1]
            for t in range(T):
                r0 = t * P
                r1 = min(r0 + P, npc)
                nt = r1 - r0
                mlo = int(plan.m_lo[t])
                mhi = int(plan.m_hi[t])
                m = mlo + mhi
                mo = int(plan.moff[t])
                pool = vpool if l == 1 else v2pool
                V = pool.tile([P, m, R], FP8 if l == 1 else BF16, tag="V",
                              name=f"V{l}_{t}")
                gather_split(V, tab[0:half, 0:R], plan.g_lo_off[t], mlo, R,
                             gidx_s)
                if mhi:
                    gather_split(V[:, mlo:m, :], tab[half:N, 0:R],
                                 plan.g_hi_off[t], mhi, R, gidx_s)

                if EDGE_LEVEL < 1:
                    continue
                mn_b = mneg_s[:, mo:mo + m].unsqueeze(2).to_broadcast(
                    [P, m, Hl])
                if l == 1:
                    Vsc = V[:, :, F:F + 2 * Hl].bitcast(BF16)
                    e_t = fpool.tile([P, m, Hl], F32, tag="e", name=f"e{l}_{t}")
                    nc.vector.tensor_tensor(out=e_t[:], in0=Vsc,
                                            in1=mn_b, op=A.add)
                    sd_b = sdst[1][t][:].unsqueeze(1).to_broadcast([P, m, Hl])
                    nc.vector.tensor_tensor(out=e_t[:], in0=e_t[:], in1=sd_b,
                                            op=A.add)
                    p_t = fpool.tile([P, m, Hl], BF16, tag="p", name=f"p{l}_{t}")
                    den = fpool.tile([P, Hl], F32, tag="den", name=f"den{l}_{t}")
                    if EDGE_LEVEL < 2:
                        continue
                    if debug_dumps and t == 0 and l == 1:
                        nc.sync.dma_start(out=dbg["e0"][:], in_=e_t[:].rearrange(
                            "p m h -> p (m h)"))
                    nc.scalar.activation(out=e_t[:], in_=e_t[:], func=ACT.Prelu,
                                         alpha=NEG)
                    for h in range(Hl):
                        nc.scalar.activation(out=p_t[:, :, h:h + 1],
                                             in_=e_t[:, :, h:h + 1],
                                             func=ACT.Exp,
                                             accum_out=den[:, h:h + 1])
                    if EDGE_LEVEL < 3:
                        continue
                    if debug_dumps and t == 0 and l == 1:
                        ptf = fpool.tile([P, m, Hl], F32, tag="ptf", name="ptf")
                        nc.vector.tensor_copy(ptf[:], p_t[:])
                        nc.sync.dma_start(out=dbg["p0"][:], in_=ptf[:].rearrange(
                            "p m h -> p (m h)"))
                else:
                    e_t = fpool.tile([P, m], F32, tag="e", name=f"e{l}_{t}")
                    nc.vector.tensor_tensor(
                        out=e_t[:], in0=V[:, :, F:F + 1].rearrange(
                            "p m o -> p (m o)"),
                        in1=mneg_s[:, mo:mo + m], op=A.add)
                    p_t = fpool.tile([P, m], BF16, tag="p", name=f"p{l}_{t}")
                    den = fpool.tile([P, 1], F32, tag="den", name=f"den{l}_{t}")
                    if EDGE_LEVEL < 2:
                        continue
                    nc.scalar.activation(out=e_t[:], in_=e_t[:], func=ACT.Prelu,
                                         bias=sdst[l][t][:, 0:1], alpha=NEG)
                    nc.scalar.activation(out=p_t[:], in_=e_t[:], func=ACT.Exp,
                                         accum_out=den[:, 0:1])
                    if EDGE_LEVEL < 3:
                        continue
                if EDGE_LEVEL < 3:
                    continue
                rc = fpool.tile([P, Hl], F32, tag="rc", name=f"rc{l}_{t}")
                nc.vector.reciprocal(rc[:], den[:])
                if debug_dumps and t == 0 and l == 1:
                    nc.sync.dma_start(out=dbg["den1"][:], in_=den[:])

                # features *= p ; U = sum over slots
                U = fpool.tile([P, F], F32, tag="U", name=f"U{l}_{t}")
                if l == 1:
                    Wt = wpool.tile([P, m, F], BF16, tag="W", name=f"W{l}_{t}")
                    v4 = V[:, :, 0:F].rearrange("p m (h f) -> p m h f", f=HID)
                    w4 = Wt[:].rearrange("p m (h f) -> p m h f", f=HID)
                    p_b = p_t[:].unsqueeze(3).to_broadcast([P, m, Hl, HID])
                    nc.vector.tensor_tensor(out=w4, in0=v4, in1=p_b, op=A.mult)
                    mm = m
                    while mm > 2:
                        a = mm // 2
                        nc.vector.tensor_tensor(
                            out=Wt[:, 0:a, :], in0=Wt[:, 0:a, :],
                            in1=Wt[:, mm - a:mm, :], op=A.add)
                        mm -= a
                    if mm == 2:
                        nc.vector.tensor_tensor(out=U[:], in0=Wt[:, 0, :],
                                                in1=Wt[:, 1, :], op=A.add)
                    else:
                        nc.vector.tensor_copy(U[:], Wt[:, 0, :])
                else:
                    v1 = V[:, :, 0:F]
                    p_b = p_t[:].unsqueeze(2).to_broadcast([P, m, F])
                    nc.vector.tensor_tensor(out=v1, in0=v1, in1=p_b, op=A.mult)
                    mm = m
                    while mm > 2:
                        a = mm // 2
                        nc.vector.tensor_tensor(
                            out=V[:, 0:a, 0:F], in0=V[:, 0:a, 0:F],
                            in1=V[:, mm - a:mm, 0:F], op=A.add)
                        mm -= a
                    if mm == 2:
                        nc.vector.tensor_tensor(out=U[:], in0=V[:, 0, 0:F],
                                                in1=V[:, 1, 0:F], op=A.add)
                    else:
                        nc.vector.tensor_copy(U[:], V[:, 0, 0:F])
                if debug_dumps and t == 0 and l == 1:
                    nc.sync.dma_start(out=dbg["U0"][:], in_=U[:])
                    nc.sync.dma_start(out=dbg["rc0"][:], in_=rc[:])

                if EDGE_LEVEL < 4:
                    continue
                # finalize: y = relu(U*rc + bbv) (+ residual)
                if l == 1:
                    x1 = fpool.tile([P, F], BF16, tag="x1", name=f"x1_{t}")
                    if triv[0]:
                        for h in range(Hl):
                            nc.scalar.activation(
                                out=x1[:, h * HID:(h + 1) * HID],
                                in_=U[:, h * HID:(h + 1) * HID],
                                func=ACT.Relu, scale=rc[:, h:h + 1])
                    else:
                        y4 = U[:].rearrange("p (h f) -> p h f", f=HID)
                        rb = rc[:].unsqueeze(2).to_broadcast([P, Hl, HID])
                        nc.vector.tensor_tensor(out=y4, in0=y4, in1=rb,
                                                op=A.mult)
                        nc.vector.tensor_tensor(out=U[:], in0=U[:],
                                                in1=bb1_s[:], op=A.add)
                        nc.vector.tensor_scalar(x1[:], U[:], 0.0, None, A.max)
                    if debug_dumps and t == 0:
                        xf = fpool.tile([P, F], F32, tag="xf", name="xf")
                        nc.vector.tensor_copy(xf[:], x1[:])
                        nc.sync.dma_start(out=dbg["x1"][:], in_=xf[:])
                    hn = x1
                else:
                    if triv[l - 1]:
                        yr = h_keep[2][t] if l == 2 else fpool.tile(
                            [P, F], BF16, tag="yr", name=f"yr{l}_{t}")
                        nc.scalar.activation(out=yr[:], in_=U[:], func=ACT.Relu,
                                             scale=rc[:, 0:1])
                    else:
                        yf = fpool.tile([P, F], F32, tag="yf", name=f"yf{l}_{t}")
                        nc.vector.scalar_tensor_tensor(
                            out=yf[:], in0=U[:], scalar=rc[:, 0:1],
                            in1=bb_s[l - 1][:], op0=A.mult, op1=A.add)
                        yr = fpool.tile([P, F], BF16, tag="yr", name=f"yr{l}_{t}")
                        nc.vector.tensor_scalar(yr[:], yf[:], 0.0, None, A.max)
                    if l == 2:
                        if yr is not h_keep[2][t]:
                            nc.vector.tensor_copy(h_keep[2][t][:], yr[:])
                        hn = h_keep[2][t]
                    elif l == 3:
                        nc.vector.tensor_tensor(out=h_keep[3][t][:], in0=yr[:],
                                                in1=h_keep[2][t][:], op=A.add)
                        hn = h_keep[3][t]
                    else:
                        h4 = fpool.tile([P, F], BF16, tag="h4", name=f"h4_{t}")
                        nc.vector.tensor_tensor(out=h4[:], in0=yr[:],
                                                in1=h_keep[3][t][:], op=A.add)
                        hn = h4
                    if debug_dumps and t == 0 and l in (2, 4):
                        hf_ = fpool.tile([P, F], F32, tag="hf_", name=f"hf{l}")
                        nc.vector.tensor_copy(hf_[:], hn[:])
                        nc.sync.dma_start(out=dbg["h2" if l == 2 else "h4"][:],
                                          in_=hf_[:])

                # next-layer table rows / pooling
                if l == 1:
                    pt2 = tpool.tile([P, HID + 2], F32, tag="tN", name=f"pt2_{t}")
                    for q in range(HF // P):
                        pT = tpool.tile([P, P], BF16, tag="tT",
                                        name=f"pT{t}_{q}")
                        nc.tensor.transpose(pT[:], x1[:, q * P:(q + 1) * P],
                                            idbf_s[:])
                        sT = spool.tile([P, P], BF16, tag="sT", name=f"sT{t}_{q}")
                        nc.scalar.activation(out=sT[:], in_=pT[:], func=ACT.Identity)
                        nc.tensor.matmul(pt2[:], lhsT=sT[:],
                                         rhs=w2p_s[:, q * (HID + 2):
                                                   (q + 1) * (HID + 2)],
                                         start=(q == 0), stop=(q == HF // P - 1))
                    tb2 = spool.tile([P, HID + 2], BF16, tag="tb2",
                                     name=f"tb2_{t}")
                    nc.scalar.activation(out=tb2[:], in_=pt2[:], func=ACT.Identity)
                    nc.scalar.activation(out=sdst[2][t][:],
                                         in_=pt2[:, HID + 1:HID + 2],
                                         func=ACT.Identity)
                    nc.sync.dma_start(out=tloc[1][r0:r1, 0:HID + 2],
                                      in_=tb2[:nt, :])
                elif l < 4:
                    pT = tpool.tile([HID, P], BF16, tag="tT", name=f"pTh{l}_{t}")
                    nc.tensor.transpose(pT[:], hn[:], idbf_s[:])
                    sT = spool.tile([HID, P], BF16, tag="sTh", name=f"sTh{l}_{t}")
                    nc.scalar.activation(out=sT[:], in_=pT[:], func=ACT.Identity)
                    ptn = tpool.tile([P, HID + 2], F32, tag="tN",
                                     name=f"ptn{l}_{t}")
                    nc.tensor.matmul(ptn[:], lhsT=sT[:], rhs=wlp_s[l][:],
                                     start=True, stop=True)
                    tbn = spool.tile([P, HID + 2], BF16, tag="tbn",
                                     name=f"tbn{l}_{t}")
                    nc.scalar.activation(out=tbn[:], in_=ptn[:], func=ACT.Identity)
                    nc.scalar.activation(out=sdst[l + 1][t][:],
                                         in_=ptn[:, HID + 1:HID + 2],
                                         func=ACT.Identity)
                    nc.sync.dma_start(out=tloc[l][r0:r1, 0:HID + 2],
                                      in_=tbn[:nt, :])
                else:
                    Sb = fpool.tile([P, B], BF16, tag="Sb", name=f"Sb_{t}")
                    bv = batchv_s[:, t:t + 1].to_broadcast([P, B])
                    nc.vector.tensor_tensor(out=Sb[:], in0=iota_s[:, 0:B],
                                            in1=bv, op=A.is_equal)
                    nc.tensor.matmul(psA[:], lhsT=hn[:], rhs=Sb[:],
                                     start=(t == 0), stop=(t == T - 1))
                    nc.tensor.matmul(psB[:], lhsT=Sb[:], rhs=ones_s[:],
                                     start=(t == 0), stop=(t == T - 1))

        edge_phase(1)
        nc.gpsimd.collective_compute("AllGather", A.bypass, replica_groups=rg,
                                     ins=[tloc[1][:]], outs=[tfull[1][:]])
        edge_phase(2)
        nc.gpsimd.collective_compute("AllGather", A.bypass, replica_groups=rg,
                                     ins=[tloc[2][:]], outs=[tfull[2][:]])
        edge_phase(3)
        nc.gpsimd.collective_compute("AllGather", A.bypass, replica_groups=rg,
                                     ins=[tloc[3][:]], outs=[tfull[3][:]])
        edge_phase(4)

        # ---------------- pooled AllReduce + MLP head (f32) ----------------
        ar_sb = cst.tile([HID, B + 1], F32, name="ar_sb", tag="ar_sb")
        nc.vector.memset(ar_sb[:], 0.0)
        nc.vector.tensor_copy(ar_sb[:, 0:B], psA[:])
        nc.vector.tensor_copy(ar_sb[0:B, B:B + 1], psB[:])
        nc.sync.dma_start(out=arin[:], in_=ar_sb[:])
        nc.gpsimd.collective_compute("AllReduce", A.add, replica_groups=rg,
                                     ins=[arin[:]], outs=[arout[:]])
        full = cst.tile([HID, B + 1], F32, name="arf", tag="arf")
        nc.sync.dma_start(out=full[:], in_=arout[:])
        cnt = cst.tile([B, 1], F32, name="cnt", tag="cnt")
        nc.vector.tensor_scalar(cnt[:], full[0:B, B:B + 1], 1.0, None, A.max)
        rcnt = cst.tile([B, 1], F32, name="rcnt", tag="rcnt")
        nc.vector.reciprocal(rcnt[:], cnt[:])
        z1p = tpool.tile([B, MH], F32, tag="tN", name="z1p")
        nc.tensor.matmul(z1p[:], lhsT=full[:, 0:B], rhs=wh1_s[:],
                         start=True, stop=True)
        z = cst.tile([B, MH], F32, name="z", tag="z")
        nc.vector.scalar_tensor_tensor(out=z[:], in0=z1p[:],
                                       scalar=rcnt[:, 0:1], in1=bh1rep_s[:],
                                       op0=A.mult, op1=A.add)
        nc.vector.tensor_scalar(z[:], z[:], 0.0, None, A.max)
        zps = tpool.tile([MH, B], F32, tag="tN", name="zps")
        nc.tensor.transpose(zps[:], z[:], idf32_s[0:B, 0:B])
        zT = cst.tile([MH, B], F32, name="zT", tag="zT")
        nc.vector.tensor_copy(zT[:], zps[:])
        ops_ = tpool.tile([B, C], F32, tag="tN", name="ops_")
        nc.tensor.matmul(ops_[:], lhsT=zT[:], rhs=wh2_s[:], start=True,
                         stop=True)
        o_sb = cst.tile([B, C], F32, name="o_sb", tag="o_sb")
        nc.vector.tensor_tensor(out=o_sb[:], in0=ops_[:], in1=bh2rep_s[:],
                                op=A.add)
        nc.sync.dma_start(out=out_d[:], in_=o_sb[:])
        _freeB()
        _freeA()

    nc.compile()
    return nc


# ----------------------------------------------------------------------------
# Runner
# ----------------------------------------------------------------------------

def run(inputs, n_cores=N_CORES, trace=False, debug_dumps=False):
    from concourse.bass_utils import run_bass_kernel_spmd
    meta, common, per_core = preprocess(inputs, n_cores=n_cores)
    nc = build_program(meta, debug_dumps=debug_dumps)
    in_maps = []
    for pc in per_core:
        m = dict(common)
        m.update(pc)
        in_maps.append(m)
    res = run_bass_kernel_spmd(nc, in_maps, list(range(n_cores)), trace=trace)
    return res


def kernel(**inputs):
    res = run(inputs)
    return np.asarray(res.results[0]["out"], np.float32)


# revision 26
# speedup vs baseline: 1.0352x; 1.0352x over previous
"""GAT (4-layer graph attention network) on 8 Trainium2 NeuronCores.

Design (v2, dst-per-partition slot layout):
  - Nodes sharded into 8 contiguous ranges; per core, local dst nodes are
    sorted by (lo-degree, hi-degree) and grouped into 128-node tiles.
  - Edges of a tile are laid out slot-major: V[d, j, :] = table row of the
    j-th source of dst d, fetched with one bulk gather per (tile, half)
    (table halves because gather indices are int16).  Pad slots gather row 0
    and are killed with a -100 score offset (exp -> ~0).
  - Per-node DRAM table rows hold [features*W' | s_src] where W' has the
    next layer's BN scale folded in; s_dst of local dsts is kept in SBUF as
    a per-partition column, added via the scalar engine's fused
    Lrelu(x + bias) -> Exp(accum_out=denominator).
  - Aggregation = DVE free-dim reduce of p-weighted gathered rows; no
    one-hot matmuls, no per-edge dst-score gather.
  - Layer 1's table (xW1, 260 cols in 384-elem rows) is built densely and
    replicated on every core; layers 2-4 build local rows and AllGather.
  - Final: per-graph mean-pool via one-hot batch matmul, AllReduce,
    replicated f32 MLP head.

kernel(**inputs) takes FULL inputs, returns the full [B, C] f32 output.
"""

import math
from contextlib import ExitStack

import numpy as np
import ml_dtypes

N_CORES = 8
NEG = 0.2
EPS = 1e-5
P = 128
DEF_SL = 2048      # dense-phase xT streaming slab columns
CHUNK_CAP = 8      # max 128-idx chunks per gather call (SWDGE limit)
MNEG = -100.0
SCALAR_COPY = True
EDGE_LEVEL = 9  # 0=gathers,1=+scores,2=+exp,3=+reduce/U,4=+finalize,9=full      # score offset for pad slots: lrelu -> -20, exp -> ~2e-9

BF = ml_dtypes.bfloat16


def cdiv(a, b):
    return -(-a // b)


# ----------------------------------------------------------------------------
# Host-side planning / preprocessing
# ----------------------------------------------------------------------------

class Plan:
    """Static program structure (cross-core maxima so SPMD program is shared).

    Per core: local dsts sorted by (lo_deg, hi_deg) desc, tiles of 128.
    m_lo[t]/m_hi[t] = max over cores of per-tile max per-dst lo/hi degree.
    """

    def __init__(self, N, E, B, IN, HID, Hh, n_cores, edge_index):
        self.N, self.E, self.B, self.IN, self.HID, self.Hh = N, E, B, IN, HID, Hh
        self.n_cores = n_cores
        self.npc = N // n_cores
        self.T = cdiv(self.npc, P)
        self.half = (n_cores // 2) * self.npc          # 25000: cores 0-3 = lo
        src = np.asarray(edge_index[0], np.int64)
        dst = np.asarray(edge_index[1], np.int64)

        npc, T = self.npc, self.T
        # Node->core assignment, two levels:
        #  1) half assignment (cores 0..3 = table rows < half) by a greedy
        #     discrepancy pass so every dst's sources split ~evenly between
        #     halves (shrinks the per-tile max-lo + max-hi slot padding);
        #  2) within each half, stride nodes over the 4 cores in total-degree
        #     order so all cores see near-identical degree profiles (keeps
        #     the shared SPMD cross-core maxima tight).
        deg = np.bincount(dst, minlength=N)
        out_deg = np.bincount(src, minlength=N)
        gorder = np.argsort(-out_deg, kind="stable")
        eo_s = np.argsort(src, kind="stable")
        dst_by_src = dst[eo_s]
        sstarts = np.searchsorted(src[eo_s], np.arange(N + 1))
        imb = np.zeros(N, np.int32)
        cap_lo = N // 2
        n_lo = 0
        n_hi = 0
        is_lo = np.zeros(N, bool)
        for n in gorder:
            ds = dst_by_src[sstarts[n]:sstarts[n + 1]]
            if (imb[ds].sum() < 0 and n_lo < cap_lo) or n_hi >= N - cap_lo:
                is_lo[n] = True
                n_lo += 1
                imb[ds] += 1
            else:
                n_hi += 1
                imb[ds] -= 1
        half_cores = n_cores // 2
        lo_nodes = np.where(is_lo)[0]
        hi_nodes = np.where(~is_lo)[0]
        lo_sorted = lo_nodes[np.argsort(-deg[lo_nodes], kind="stable")]
        hi_sorted = hi_nodes[np.argsort(-deg[hi_nodes], kind="stable")]
        self.node_of = [lo_sorted[c::half_cores] for c in range(half_cores)] \
            + [hi_sorted[c::half_cores] for c in range(half_cores)]
        core_of = np.empty(N, np.int64)
        for c in range(n_cores):
            core_of[self.node_of[c]] = c

        self.adj = []             # per core: list over tiles of (lo_lists, hi_lists)
        m_lo = np.zeros((n_cores, T), np.int64)
        m_hi = np.zeros((n_cores, T), np.int64)
        src_lo = is_lo[src]
        for c in range(n_cores):
            mine = core_of[dst] == c
            s_c = src[mine]
            slo_c = src_lo[mine]
            # local dst index within this core's preliminary order
            lidx = np.empty(N, np.int64)
            lidx[self.node_of[c]] = np.arange(npc)
            d_c = lidx[dst[mine]]
            lo_cnt = np.bincount(d_c[slo_c], minlength=npc)
            hi_cnt = np.bincount(d_c[~slo_c], minlength=npc)
            perm = np.lexsort((-hi_cnt, -lo_cnt))
            self.node_of[c] = self.node_of[c][perm]     # final tile order
            eo = np.argsort(d_c, kind="stable")
            s_sorted = s_c[eo]
            slo_sorted = slo_c[eo]
            starts = np.searchsorted(d_c[eo], np.arange(npc + 1))
            tiles = []
            for t in range(T):
                lo_lists, hi_lists = [], []
                for d in perm[t * P: (t + 1) * P]:
                    e = s_sorted[starts[d]:starts[d + 1]]
                    sl = slo_sorted[starts[d]:starts[d + 1]]
                    lo_lists.append(e[sl])
                    hi_lists.append(e[~sl])
                if len(lo_lists) < P:       # pad partial tile
                    pad = P - len(lo_lists)
                    lo_lists += [np.empty(0, np.int64)] * pad
                    hi_lists += [np.empty(0, np.int64)] * pad
                tiles.append((lo_lists, hi_lists))
                m_lo[c, t] = max(len(x) for x in lo_lists)
                m_hi[c, t] = max(len(x) for x in hi_lists)
            self.adj.append(tiles)
        self.m_lo = np.maximum(m_lo.max(axis=0), 1).astype(np.int64)
        self.m_hi = m_hi.max(axis=0).astype(np.int64)
        self.m_tot = self.m_lo + self.m_hi
        # column offsets into mneg table [P, sum(m_tot)]
        self.moff = np.concatenate([[0], np.cumsum(self.m_tot)]).astype(np.int64)
        self.MCOLS = int(self.moff[-1])
        # gidx column layout: per tile: 8*m_lo cols then 8*m_hi cols
        self.g_lo_off = []
        self.g_hi_off = []
        go = 0
        for t in range(T):
            self.g_lo_off.append(go)
            go += 8 * int(self.m_lo[t])
            self.g_hi_off.append(go)
            go += 8 * int(self.m_hi[t])
        self.GCOLS = go
        # global permuted position of each node (gather index base):
        # node_of[c][i] sits at table row c*npc + i
        gpos = np.empty(N, np.int64)
        for c in range(n_cores):
            gpos[self.node_of[c]] = c * npc + np.arange(npc)
        self.gpos = gpos


def _wrap16(vals16):
    """[n] -> [128, n/16] int16: 16-partition-wrapped, replicated x8."""
    n = vals16.shape[0]
    assert n % 16 == 0
    a = vals16.reshape(n // 16, 16).T.astype(np.int16)
    return np.tile(a, (8, 1))


def preprocess(inputs, n_cores=N_CORES):
    x = np.asarray(inputs["x"], np.float32)
    edge_index = np.asarray(inputs["edge_index"])
    batch = np.asarray(inputs["batch"], np.int64)
    N, IN = x.shape
    E = edge_index.shape[1]
    a_src1 = np.asarray(inputs["a_src1"], np.float32)
    Hh, HID = a_src1.shape
    C = np.asarray(inputs["Wh2"], np.float32).shape[1]
    B = 64 if N == 50000 else int(batch.max()) + 1

    plan = Plan(N, E, B, IN, HID, Hh, n_cores, edge_index)
    npc, T, half = plan.npc, plan.T, plan.half

    HF = Hh * HID                 # 256
    R1 = 512                      # layer-1 fp8 row elems (256 fp8 + 4 bf16 scores)
    R2 = 128                      # layer 2-4 table row elems (65 used)
    gs = 1.0 / math.sqrt(1.0 + EPS)

    def fold(W, a_s, a_d, g=None):
        """[W*diag(g)*gs | W@a_s per head | W@a_d per head]"""
        W = np.asarray(W, np.float32)
        a_s = np.asarray(a_s, np.float32)
        a_d = np.asarray(a_d, np.float32)
        Fin = W.shape[0]
        hh, F = a_s.shape
        Wr = W.reshape(Fin, hh, F)
        ws = np.einsum("ihf,hf->ih", Wr, a_s)
        wd = np.einsum("ihf,hf->ih", Wr, a_d)
        Wsc = W if g is None else W * (np.asarray(g, np.float32) * gs)[None, :]
        return np.concatenate([Wsc, ws, wd], axis=1).astype(BF)

    w1p = fold(inputs["W1"], a_src1, inputs["a_dst1"])                # [128,264]
    w2p = fold(inputs["W2"], inputs["a_src2"], inputs["a_dst2"],
               inputs["g2"])                                          # [256,66]
    nq2 = HF // P
    w2p = np.concatenate([w2p[q * P:(q + 1) * P, :] for q in range(nq2)],
                         axis=1)                                      # [128,132]
    w3p = fold(inputs["W3"], inputs["a_src3"], inputs["a_dst3"], inputs["g3"])
    w4p = fold(inputs["W4"], inputs["a_src4"], inputs["a_dst4"], inputs["g4"])

    # bias vectors after BN fold: y = U'*rc + bbv ; scalar path iff bbv == 0
    def bbv(g, b, be):
        return (np.asarray(g, np.float32) * gs * np.asarray(b, np.float32)
                + np.asarray(be, np.float32))

    b1 = np.asarray(inputs["b1"], np.float32)
    bb = [b1, bbv(inputs["g2"], inputs["b2"], inputs["be2"]),
          bbv(inputs["g3"], inputs["b3"], inputs["be3"]),
          bbv(inputs["g4"], inputs["b4"], inputs["be4"])]
    triv = [bool(np.all(np.abs(v) < 1e-30)) for v in bb]
    bbrep = [np.tile(v[None, :], (P, 1)).astype(np.float32) for v in bb]

    wh1 = np.asarray(inputs["Wh1"], np.float32)
    MH = wh1.shape[1]
    bh1rep = np.tile(np.asarray(inputs["bh1"], np.float32)[None, :], (B, 1))
    wh2 = np.asarray(inputs["Wh2"], np.float32)
    bh2rep = np.tile(np.asarray(inputs["bh2"], np.float32)[None, :], (B, 1))

    # globally permuted x^T for the dense phase (every core builds full table1)
    permg = np.concatenate([plan.node_of[c] for c in range(n_cores)])
    xTg = np.ascontiguousarray(x[permg].T).astype(BF)                 # [IN, N]
    idbf = np.eye(P, dtype=np.float32).astype(BF)
    idf32 = np.eye(P, dtype=np.float32)
    iota = np.tile(np.arange(P, dtype=np.float32)[None, :], (P, 1)).astype(BF)
    onescol = np.ones((P, 1), np.float32).astype(BF)

    common = dict(xTg=xTg, w1p=w1p, w2p=w2p, w3p=w3p, w4p=w4p,
                  bb1=bbrep[0][:, :HF], bb2=bbrep[1], bb3=bbrep[2],
                  bb4=bbrep[3], wh1=wh1, bh1rep=bh1rep, wh2=wh2,
                  bh2rep=bh2rep, idbf=idbf, idf32=idf32, iota=iota,
                  onescol=onescol)

    per_core = []
    for c in range(n_cores):
        gidx = np.zeros((P, max(plan.GCOLS, 8)), np.int16)
        mneg = np.full((P, max(plan.MCOLS, 1)), MNEG, np.float32)
        for t in range(T):
            lo_lists, hi_lists = plan.adj[c][t]
            for which, lists, m, goff in (
                    (0, lo_lists, int(plan.m_lo[t]), plan.g_lo_off[t]),
                    (1, hi_lists, int(plan.m_hi[t]), plan.g_hi_off[t])):
                if m == 0:
                    continue
                iv = np.zeros((P, m), np.int16)       # [dst, slot]
                for d, e in enumerate(lists):
                    ge = plan.gpos[e] - (half if which else 0)
                    iv[d, :len(e)] = ge.astype(np.int16)
                    if which == 0:
                        mneg[d, plan.moff[t]:plan.moff[t] + len(e)] = 0.0
                    else:
                        o = plan.moff[t] + int(plan.m_lo[t])
                        mneg[d, o:o + len(e)] = 0.0
                # slot-major: position j*128+d  -> value iv[d, j]
                vals = iv.T.reshape(-1)               # [m*128]
                gidx[:, goff:goff + 8 * m] = _wrap16(vals)

        batchv = np.full((P, T), -1.0, np.float32)
        bperm = batch[plan.node_of[c]].astype(np.float32)
        for t in range(T):
            nt = min((t + 1) * P, npc) - t * P
            batchv[:nt, t] = bperm[t * P:t * P + nt]

        xtl = np.zeros((IN, T * P), BF)
        xtl[:, :npc] = x[plan.node_of[c]].T.astype(BF)
        per_core.append(dict(gidx=gidx, mneg=mneg.astype(BF),
                             batchv=batchv.astype(BF), xtl=xtl))

    meta = dict(plan=plan, HF=HF, R1=R1, R2=R2, C=C, MH=MH, B=B, triv=triv)
    return meta, common, per_core


# ----------------------------------------------------------------------------
# Bass program (shared by all cores; per-core behavior differs only via data)
# ----------------------------------------------------------------------------

def build_program(meta, debug_dumps=False):
    import concourse.bass as bass
    import concourse.bacc as bacc
    import concourse.mybir as mybir
    import concourse.tile as tile

    F32 = mybir.dt.float32
    BF16 = mybir.dt.bfloat16
    FP8 = mybir.dt.float8e4
    I16 = mybir.dt.int16
    A = mybir.AluOpType
    ACT = mybir.ActivationFunctionType
    X = mybir.AxisListType.X

    plan = meta["plan"]
    N, IN, Hh, HID = plan.N, plan.IN, plan.Hh, plan.HID
    B, C, MH = meta["B"], meta["C"], meta["MH"]
    HF, R1, R2 = meta["HF"], meta["R1"], meta["R2"]
    triv = meta["triv"]
    npc, T, half = plan.npc, plan.T, plan.half
    n_cores = plan.n_cores
    SL = min(DEF_SL, N)

    nc = bacc.Bacc("TRN2", num_devices=n_cores, num_swdge_queues=4)
    rg = [list(range(n_cores))]

    def ein(name, shape, dt):
        return nc.dram_tensor(name, shape, dt, kind="ExternalInput")

    xTg_d = ein("xTg", [IN, N], BF16)
    xtl_d = ein("xtl", [IN, T * P], BF16)
    w1p_d = ein("w1p", [IN, HF + 2 * Hh], BF16)
    w2p_d = ein("w2p", [P, (HF // P) * (HID + 2)], BF16)
    w3p_d = ein("w3p", [HID, HID + 2], BF16)
    w4p_d = ein("w4p", [HID, HID + 2], BF16)
    bb1_d = ein("bb1", [P, HF], F32)
    bb_d = [None, ein("bb2", [P, HID], F32), ein("bb3", [P, HID], F32),
            ein("bb4", [P, HID], F32)]
    wh1_d = ein("wh1", [HID, MH], F32)
    bh1rep_d = ein("bh1rep", [B, MH], F32)
    wh2_d = ein("wh2", [MH, C], F32)
    bh2rep_d = ein("bh2rep", [B, C], F32)
    idbf_d = ein("idbf", [P, P], BF16)
    idf32_d = ein("idf32", [P, P], F32)
    iota_d = ein("iota", [P, P], BF16)
    ones_d = ein("onescol", [P, 1], BF16)
    gidx_d = ein("gidx", [P, max(plan.GCOLS, 8)], I16)
    mneg_d = ein("mneg", [P, max(plan.MCOLS, 1)], BF16)
    batchv_d = ein("batchv", [P, T], BF16)

    shr = "Shared" if n_cores > 4 else "Local"
    table1 = nc.dram_tensor("table1", [N, R1], FP8)
    tloc = [None, nc.dram_tensor("tloc2", [npc, R2], BF16),
            nc.dram_tensor("tloc3", [npc, R2], BF16),
            nc.dram_tensor("tloc4", [npc, R2], BF16)]
    tfull = [None,
             nc.dram_tensor("tfull2", [N, R2], BF16, addr_space=shr),
             nc.dram_tensor("tfull3", [N, R2], BF16, addr_space=shr),
             nc.dram_tensor("tfull4", [N, R2], BF16, addr_space=shr)]
    arin = nc.dram_tensor("arin", [HID, B + 1], F32)
    arout = nc.dram_tensor("arout", [HID, B + 1], F32, addr_space=shr)
    out_d = nc.dram_tensor("out", [B, C], F32, kind="ExternalOutput")
    dbg = {}
    if debug_dumps:
        dbg["x1"] = nc.dram_tensor("dbg_x1", [P, HF], F32, kind="ExternalOutput")
        dbg["den1"] = nc.dram_tensor("dbg_den1", [P, Hh], F32, kind="ExternalOutput")
        dbg["h2"] = nc.dram_tensor("dbg_h2", [P, HID], F32, kind="ExternalOutput")
        M0 = int(plan.m_tot[0])
        dbg["e0"] = nc.dram_tensor("dbg_e0", [P, M0 * Hh], F32, kind="ExternalOutput")
        dbg["U0"] = nc.dram_tensor("dbg_U0", [P, HF], F32, kind="ExternalOutput")
        dbg["rc0"] = nc.dram_tensor("dbg_rc0", [P, Hh], F32, kind="ExternalOutput")
        dbg["p0"] = nc.dram_tensor("dbg_p0", [P, M0 * Hh], F32, kind="ExternalOutput")
        dbg["h4"] = nc.dram_tensor("dbg_h4", [P, HID], F32, kind="ExternalOutput")

    gcnt = nc.gpsimd.alloc_register("gcnt")
    qctr = [0]

    def gather_split(out3, tab_ap, col0, n_chunks, elem, gidx_s):
        done = 0
        ncalls = cdiv(n_chunks, CHUNK_CAP)
        per = cdiv(n_chunks, max(ncalls, 1))   # balanced call sizes
        while done < n_chunks:
            nn = min(per, n_chunks - done)
            nc.gpsimd.reg_mov(gcnt, nn * P)
            nc.gpsimd.dma_gather(
                out3[:, done:done + nn, :], tab_ap,
                gidx_s[:, col0 + 8 * done: col0 + 8 * (done + nn)],
                nn * P, gcnt, elem, queue_num=qctr[0] % 4)
            qctr[0] += 1
            done += nn

    with ExitStack() as ctx:
        tc = ctx.enter_context(tile.TileContext(nc))
        cst = ctx.enter_context(tc.tile_pool(name="cst", bufs=1))
        vpool = ctx.enter_context(tc.tile_pool(name="vpool", bufs=2))
        v2pool = ctx.enter_context(tc.tile_pool(name="v2pool", bufs=4))
        wpool = ctx.enter_context(tc.tile_pool(name="wpool", bufs=1))
        fpool = ctx.enter_context(tc.tile_pool(name="fpool", bufs=2))
        spool = ctx.enter_context(tc.tile_pool(name="spool", bufs=2))
        xpool = ctx.enter_context(tc.tile_pool(name="xpool", bufs=2))
        hpool = ctx.enter_context(tc.tile_pool(name="hpool", bufs=1))
        tpool = ctx.enter_context(tc.tile_pool(name="tpool", bufs=2, space="PSUM"))

        def load_const(dram, shape, dt, name):
            t = cst.tile(shape, dt, name=name, tag=name)
            nc.sync.dma_start(out=t[:], in_=dram[:])
            return t

        w1p_s = load_const(w1p_d, [IN, HF + 2 * Hh], BF16, "w1p_s")
        w2p_s = load_const(w2p_d, [P, (HF // P) * (HID + 2)], BF16, "w2p_s")
        w3p_s = load_const(w3p_d, [HID, HID + 2], BF16, "w3p_s")
        w4p_s = load_const(w4p_d, [HID, HID + 2], BF16, "w4p_s")
        wlp_s = [None, w2p_s, w3p_s, w4p_s]
        bb1_s = load_const(bb1_d, [P, HF], F32, "bb1_s")
        bb_s = [None] + [load_const(bb_d[i], [P, HID], F32, f"bb{i+1}_s")
                         for i in (1, 2, 3)]
        wh1_s = load_const(wh1_d, [HID, MH], F32, "wh1_s")
        bh1rep_s = load_const(bh1rep_d, [B, MH], F32, "bh1rep_s")
        wh2_s = load_const(wh2_d, [MH, C], F32, "wh2_s")
        bh2rep_s = load_const(bh2rep_d, [B, C], F32, "bh2rep_s")
        idbf_s = load_const(idbf_d, [P, P], BF16, "idbf_s")
        idf32_s = load_const(idf32_d, [P, P], F32, "idf32_s")
        iota_s = load_const(iota_d, [P, P], BF16, "iota_s")
        ones_s = load_const(ones_d, [P, 1], BF16, "ones_s")
        gidx_s = load_const(gidx_d, [P, max(plan.GCOLS, 8)], I16, "gidx_s")
        mneg_s = load_const(mneg_d, [P, max(plan.MCOLS, 1)], BF16, "mneg_s")
        batchv_s = load_const(batchv_d, [P, T], BF16, "batchv_s")
        xtl_s = load_const(xtl_d, [IN, T * P], BF16, "xtl_s")

        # persistent per-tile state
        sdst = {1: [], 2: [], 3: [], 4: []}   # [P, Hl] f32 per tile (layer l)
        h_keep = {2: [], 3: []}
        for t in range(T):
            sdst[1].append(hpool.tile([P, Hh], F32, tag=f"sd1_{t}",
                                      name=f"sd1_{t}"))
            for l in (2, 3, 4):
                sdst[l].append(hpool.tile([P, 1], F32, tag=f"sd{l}_{t}",
                                          name=f"sd{l}_{t}"))
            h_keep[2].append(hpool.tile([P, HID], BF16, tag=f"h2_{t}",
                                        name=f"h2_{t}"))
            h_keep[3].append(hpool.tile([P, HID], BF16, tag=f"h3_{t}",
                                        name=f"h3_{t}"))

        psA, _freeA = tc.tile([HID, B], F32, space="PSUM", name="psA")
        psB, _freeB = tc.tile([B, 1], F32, space="PSUM", name="psB")

        # ---------------- dense phase: table1 rows (replicated, permuted) ---
        for sb in range(cdiv(N, SL)):
            c0 = sb * SL
            c1 = min(c0 + SL, N)
            xsl = xpool.tile([IN, c1 - c0], BF16, tag="xsl", name=f"xsl{sb}")
            nc.sync.dma_start(out=xsl[:], in_=xTg_d[:, c0:c1])
            for blk in range(c0 // P, cdiv(c1, P)):
                b0 = blk * P
                b1_ = min(b0 + P, N)
                nb = b1_ - b0
                ps = tpool.tile([P, HF + 2 * Hh], F32, tag="tN",
                                name=f"psd{blk}")
                nc.tensor.matmul(ps[:nb, :], lhsT=xsl[:, b0 - c0:b1_ - c0],
                                 rhs=w1p_s[:], start=True, stop=True)
                tb = spool.tile([P, HF + 2 * Hh], FP8, tag="tbd",
                                name=f"tbd{blk}")
                if blk % 2 == 0 or not SCALAR_COPY:
                    nc.vector.tensor_copy(tb[:nb, 0:HF], ps[:nb, 0:HF])
                else:
                    nc.scalar.activation(out=tb[:nb, 0:HF], in_=ps[:nb, 0:HF],
                                         func=ACT.Identity)
                nc.vector.tensor_copy(
                    tb[:nb, HF:HF + 2 * Hh].bitcast(BF16),
                    ps[:nb, HF:HF + Hh])
                nc.sync.dma_start(out=table1[b0:b1_, 0:HF + 2 * Hh],
                                  in_=tb[:nb, :])
        # local s_dst for layer 1 (from zero-padded local xT)
        for t in range(T):
            psd = tpool.tile([P, Hh], F32, tag="tN", name=f"psd2_{t}")
            nc.tensor.matmul(psd[:], lhsT=xtl_s[:, t * P:(t + 1) * P],
                             rhs=w1p_s[:, HF + Hh:HF + 2 * Hh],
                             start=True, stop=True)
            if SCALAR_COPY:
                nc.scalar.activation(out=sdst[1][t][:], in_=psd[:],
                                     func=ACT.Identity)
            else:
                nc.vector.tensor_copy(sdst[1][t][:], psd[:])

        # ---------------- edge phase ----------------
        def edge_phase(l):
            R = R1 if l == 1 else R2
            F = HF if l == 1 else HID
            Hl = Hh if l == 1 else 1
            tab = table1 if l == 1 else tfull[l - 1]
            for t in range(T):
                r0 = t * P
                r1 = min(r0 + P, npc)
                nt = r1 - r0
                mlo = int(plan.m_lo[t])
                mhi = int(plan.m_hi[t])
                m = mlo + mhi
                mo = int(plan.moff[t])
                pool = vpool if l == 1 else v2pool
                V = pool.tile([P, m, R], FP8 if l == 1 else BF16, tag="V",
                              name=f"V{l}_{t}")
                gather_split(V, tab[0:half, 0:R], plan.g_lo_off[t], mlo, R,
                             gidx_s)
                if mhi:
                    gather_split(V[:, mlo:m, :], tab[half:N, 0:R],
                                 plan.g_hi_off[t], mhi, R, gidx_s)

                if EDGE_LEVEL < 1:
                    continue
                mn_b = mneg_s[:, mo:mo + m].unsqueeze(2).to_broadcast(
                    [P, m, Hl])
                if l == 1:
                    Vsc = V[:, :, F:F + 2 * Hl].bitcast(BF16)
                    e_t = fpool.tile([P, m, Hl], F32, tag="e", name=f"e{l}_{t}")
                    nc.vector.tensor_tensor(out=e_t[:], in0=Vsc,
                                            in1=mn_b, op=A.add)
                    sd_b = sdst[1][t][:].unsqueeze(1).to_broadcast([P, m, Hl])
                    nc.vector.tensor_tensor(out=e_t[:], in0=e_t[:], in1=sd_b,
                                            op=A.add)
                    p_t = fpool.tile([P, m, Hl], BF16, tag="p", name=f"p{l}_{t}")
                    den = fpool.tile([P, Hl], F32, tag="den", name=f"den{l}_{t}")
                    if EDGE_LEVEL < 2:
                        continue
                    if debug_dumps and t == 0 and l == 1:
                        nc.sync.dma_start(out=dbg["e0"][:], in_=e_t[:].rearrange(
                            "p m h -> p (m h)"))
                    nc.scalar.activation(out=e_t[:], in_=e_t[:], func=ACT.Prelu,
                                         alpha=NEG)
                    for h in range(Hl):
                        nc.scalar.activation(out=p_t[:, :, h:h + 1],
                                             in_=e_t[:, :, h:h + 1],
                                             func=ACT.Exp,
                                             accum_out=den[:, h:h + 1])
                    if EDGE_LEVEL < 3:
                        continue
                    if debug_dumps and t == 0 and l == 1:
                        ptf = fpool.tile([P, m, Hl], F32, tag="ptf", name="ptf")
                        nc.vector.tensor_copy(ptf[:], p_t[:])
                        nc.sync.dma_start(out=dbg["p0"][:], in_=ptf[:].rearrange(
                            "p m h -> p (m h)"))
                else:
                    e_t = fpool.tile([P, m], F32, tag="e", name=f"e{l}_{t}")
                    nc.vector.tensor_tensor(
                        out=e_t[:], in0=V[:, :, F:F + 1].rearrange(
                            "p m o -> p (m o)"),
                        in1=mneg_s[:, mo:mo + m], op=A.add)
                    p_t = fpool.tile([P, m], BF16, tag="p", name=f"p{l}_{t}")
                    den = fpool.tile([P, 1], F32, tag="den", name=f"den{l}_{t}")
                    if EDGE_LEVEL < 2:
                        continue
                    nc.scalar.activation(out=e_t[:], in_=e_t[:], func=ACT.Prelu,
                                         bias=sdst[l][t][:, 0:1], alpha=NEG)
                    nc.scalar.activation(out=p_t[:], in_=e_t[:], func=ACT.Exp,
                                         accum_out=den[:, 0:1])
                    if EDGE_LEVEL < 3:
                        continue
                if EDGE_LEVEL < 3:
                    continue
                rc = fpool.tile([P, Hl], F32, tag="rc", name=f"rc{l}_{t}")
                nc.vector.reciprocal(rc[:], den[:])
                if debug_dumps and t == 0 and l == 1:
                    nc.sync.dma_start(out=dbg["den1"][:], in_=den[:])

                # features *= p ; U = sum over slots
                U = fpool.tile([P, F], F32, tag="U", name=f"U{l}_{t}")
                if l == 1:
                    Wt = wpool.tile([P, m, F], BF16, tag="W", name=f"W{l}_{t}")
                    v4 = V[:, :, 0:F].rearrange("p m (h f) -> p m h f", f=HID)
                    w4 = Wt[:].rearrange("p m (h f) -> p m h f", f=HID)
                    p_b = p_t[:].unsqueeze(3).to_broadcast([P, m, Hl, HID])
                    nc.vector.tensor_tensor(out=w4, in0=v4, in1=p_b, op=A.mult)
                    mm = m
                    while mm > 2:
                        a = mm // 2
                        nc.vector.tensor_tensor(
                            out=Wt[:, 0:a, :], in0=Wt[:, 0:a, :],
                            in1=Wt[:, mm - a:mm, :], op=A.add)
                        mm -= a
                    if mm == 2:
                        nc.vector.tensor_tensor(out=U[:], in0=Wt[:, 0, :],
                                                in1=Wt[:, 1, :], op=A.add)
                    else:
                        nc.vector.tensor_copy(U[:], Wt[:, 0, :])
                else:
                    v1 = V[:, :, 0:F]
                    p_b = p_t[:].unsqueeze(2).to_broadcast([P, m, F])
                    nc.vector.tensor_tensor(out=v1, in0=v1, in1=p_b, op=A.mult)
                    nc.vector.tensor_reduce(U[:], V[:, 0:m, 0:F].transpose(
                        [0, 2, 1]), X, A.add)
                if debug_dumps and t == 0 and l == 1:
                    nc.sync.dma_start(out=dbg["U0"][:], in_=U[:])
                    nc.sync.dma_start(out=dbg["rc0"][:], in_=rc[:])

                if EDGE_LEVEL < 4:
                    continue
                # finalize: y = relu(U*rc + bbv) (+ residual)
                if l == 1:
                    x1 = fpool.tile([P, F], BF16, tag="x1", name=f"x1_{t}")
                    if triv[0]:
                        for h in range(Hl):
                            nc.scalar.activation(
                                out=x1[:, h * HID:(h + 1) * HID],
                                in_=U[:, h * HID:(h + 1) * HID],
                                func=ACT.Relu, scale=rc[:, h:h + 1])
                    else:
                        y4 = U[:].rearrange("p (h f) -> p h f", f=HID)
                        rb = rc[:].unsqueeze(2).to_broadcast([P, Hl, HID])
                        nc.vector.tensor_tensor(out=y4, in0=y4, in1=rb,
                                                op=A.mult)
                        nc.vector.tensor_tensor(out=U[:], in0=U[:],
                                                in1=bb1_s[:], op=A.add)
                        nc.vector.tensor_scalar(x1[:], U[:], 0.0, None, A.max)
                    if debug_dumps and t == 0:
                        xf = fpool.tile([P, F], F32, tag="xf", name="xf")
                        nc.vector.tensor_copy(xf[:], x1[:])
                        nc.sync.dma_start(out=dbg["x1"][:], in_=xf[:])
                    hn = x1
                else:
                    if triv[l - 1]:
                        yr = h_keep[2][t] if l == 2 else fpool.tile(
                            [P, F], BF16, tag="yr", name=f"yr{l}_{t}")
                        nc.scalar.activation(out=yr[:], in_=U[:], func=ACT.Relu,
                                             scale=rc[:, 0:1])
                    else:
                        yf = fpool.tile([P, F], F32, tag="yf", name=f"yf{l}_{t}")
                        nc.vector.scalar_tensor_tensor(
                            out=yf[:], in0=U[:], scalar=rc[:, 0:1],
                            in1=bb_s[l - 1][:], op0=A.mult, op1=A.add)
                        yr = fpool.tile([P, F], BF16, tag="yr", name=f"yr{l}_{t}")
                        nc.vector.tensor_scalar(yr[:], yf[:], 0.0, None, A.max)
                    if l == 2:
                        if yr is not h_keep[2][t]:
                            nc.vector.tensor_copy(h_keep[2][t][:], yr[:])
                        hn = h_keep[2][t]
                    elif l == 3:
                        nc.vector.tensor_tensor(out=h_keep[3][t][:], in0=yr[:],
                                                in1=h_keep[2][t][:], op=A.add)
                        hn = h_keep[3][t]
                    else:
                        h4 = fpool.tile([P, F], BF16, tag="h4", name=f"h4_{t}")
                        nc.vector.tensor_tensor(out=h4[:], in0=yr[:],
                                                in1=h_keep[3][t][:], op=A.add)
                        hn = h4
                    if debug_dumps and t == 0 and l in (2, 4):
                        hf_ = fpool.tile([P, F], F32, tag="hf_", name=f"hf{l}")
                        nc.vector.tensor_copy(hf_[:], hn[:])
                        nc.sync.dma_start(out=dbg["h2" if l == 2 else "h4"][:],
                                          in_=hf_[:])

                # next-layer table rows / pooling
                if l == 1:
                    pt2 = tpool.tile([P, HID + 2], F32, tag="tN", name=f"pt2_{t}")
                    for q in range(HF // P):
                        pT = tpool.tile([P, P], BF16, tag="tT",
                                        name=f"pT{t}_{q}")
                        nc.tensor.transpose(pT[:], x1[:, q * P:(q + 1) * P],
                                            idbf_s[:])
                        sT = spool.tile([P, P], BF16, tag="sT", name=f"sT{t}_{q}")
                        nc.scalar.activation(out=sT[:], in_=pT[:], func=ACT.Identity)
                        nc.tensor.matmul(pt2[:], lhsT=sT[:],
                                         rhs=w2p_s[:, q * (HID + 2):
                                                   (q + 1) * (HID + 2)],
                                         start=(q == 0), stop=(q == HF // P - 1))
                    tb2 = spool.tile([P, HID + 2], BF16, tag="tb2",
                                     name=f"tb2_{t}")
                    nc.scalar.activation(out=tb2[:], in_=pt2[:], func=ACT.Identity)
                    nc.scalar.activation(out=sdst[2][t][:],
                                         in_=pt2[:, HID + 1:HID + 2],
                                         func=ACT.Identity)
                    nc.sync.dma_start(out=tloc[1][r0:r1, 0:HID + 2],
                                      in_=tb2[:nt, :])
                elif l < 4:
                    pT = tpool.tile([HID, P], BF16, tag="tT", name=f"pTh{l}_{t}")
                    nc.tensor.transpose(pT[:], hn[:], idbf_s[:])
                    sT = spool.tile([HID, P], BF16, tag="sTh", name=f"sTh{l}_{t}")
                    nc.scalar.activation(out=sT[:], in_=pT[:], func=ACT.Identity)
                    ptn = tpool.tile([P, HID + 2], F32, tag="tN",
                                     name=f"ptn{l}_{t}")
                    nc.tensor.matmul(ptn[:], lhsT=sT[:], rhs=wlp_s[l][:],
                                     start=True, stop=True)
                    tbn = spool.tile([P, HID + 2], BF16, tag="tbn",
                                     name=f"tbn{l}_{t}")
                    nc.scalar.activation(out=tbn[:], in_=ptn[:], func=ACT.Identity)
                    nc.scalar.activation(out=sdst[l + 1][t][:],
                                         in_=ptn[:, HID + 1:HID + 2],
                                         func=ACT.Identity)
                    nc.sync.dma_start(out=tloc[l][r0:r1, 0:HID + 2],
                                      in_=tbn[:nt, :])
                else:
                    Sb = fpool.tile([P, B], BF16, tag="Sb", name=f"Sb_{t}")
                    bv = batchv_s[:, t:t + 1].to_broadcast([P, B])
                    nc.vector.tensor_tensor(out=Sb[:], in0=iota_s[:, 0:B],
                                            in1=bv, op=A.is_equal)
                    nc.tensor.matmul(psA[:], lhsT=hn[:], rhs=Sb[:],
                                     start=(t == 0), stop=(t == T - 1))
                    nc.tensor.matmul(psB[:], lhsT=Sb[:], rhs=ones_s[:],
                                     start=(t == 0), stop=(t == T - 1))

        edge_phase(1)
        nc.gpsimd.collective_compute("AllGather", A.bypass, replica_groups=rg,
                                     ins=[tloc[1][:]], outs=[tfull[1][:]])
        edge_phase(2)
        nc.gpsimd.collective_compute("AllGather", A.bypass, replica_groups=rg,
                                     ins=[tloc[2][:]], outs=[tfull[2][:]])
        edge_phase(3)
        nc.gpsimd.collective_compute("AllGather", A.bypass, replica_groups=rg,
                                     ins=[tloc[3][:]], outs=[tfull[3][:]])
        edge_phase(4)

        # ---------------- pooled AllReduce + MLP head (f32) ----------------
        ar_sb = cst.tile([HID, B + 1], F32, name="ar_sb", tag="ar_sb")
        nc.vector.memset(ar_sb[:], 0.0)
        nc.vector.tensor_copy(ar_sb[:, 0:B], psA[:])
        nc.vector.tensor_copy(ar_sb[0:B, B:B + 1], psB[:])
        nc.sync.dma_start(out=arin[:], in_=ar_sb[:])
        nc.gpsimd.collective_compute("AllReduce", A.add, replica_groups=rg,
                                     ins=[arin[:]], outs=[arout[:]])
        full = cst.tile([HID, B + 1], F32, name="arf", tag="arf")
        nc.sync.dma_start(out=full[:], in_=arout[:])
        cnt = cst.tile([B, 1], F32, name="cnt", tag="cnt")
        nc.vector.tensor_scalar(cnt[:], full[0:B, B:B + 1], 1.0, None, A.max)
        rcnt = cst.tile([B, 1], F32, name="rcnt", tag="rcnt")
        nc.vector.reciprocal(rcnt[:], cnt[:])
        z1p = tpool.tile([B, MH], F32, tag="tN", name="z1p")
        nc.tensor.matmul(z1p[:], lhsT=full[:, 0:B], rhs=wh1_s[:],
                         start=True, stop=True)
        z = cst.tile([B, MH], F32, name="z", tag="z")
        nc.vector.scalar_tensor_tensor(out=z[:], in0=z1p[:],
                                       scalar=rcnt[:, 0:1], in1=bh1rep_s[:],
                                       op0=A.mult, op1=A.add)
        nc.vector.tensor_scalar(z[:], z[:], 0.0, None, A.max)
        zps = tpool.tile([MH, B], F32, tag="tN", name="zps")
        nc.tensor.transpose(zps[:], z[:], idf32_s[0:B, 0:B])
        zT = cst.tile([MH, B], F32, name="zT", tag="zT")
        nc.vector.tensor_copy(zT[:], zps[:])
        ops_ = tpool.tile([B, C], F32, tag="tN", name="ops_")
        nc.tensor.matmul(ops_[:], lhsT=zT[:], rhs=wh2_s[:], start=True,
                         stop=True)
        o_sb = cst.tile([B, C], F32, name="o_sb", tag="o_sb")
        nc.vector.tensor_tensor(out=o_sb[:], in0=ops_[:], in1=bh2rep_s[:],
                                op=A.add)
        nc.sync.dma_start(out=out_d[:], in_=o_sb[:])
        _freeB()
        _freeA()

    nc.compile()
    return nc


# ----------------------------------------------------------------------------
# Runner
# ----------------------------------------------------------------------------

def run(inputs, n_cores=N_CORES, trace=False, debug_dumps=False):
    from concourse.bass_utils import run_bass_kernel_spmd
    meta, common, per_core = preprocess(inputs, n_cores=n_cores)
    nc = build_program(meta, debug_dumps=debug_dumps)
    in_maps = []
    for pc in per_core:
        m = dict(common)
        m.update(pc)
        in_maps.append(m)
    res = run_bass_kernel_spmd(nc, in_maps, list(range(n_cores)), trace=trace)
    return res


def kernel(**inputs):
    res = run(inputs)
    return np.asarray(res.results[0]["out"], np.float32)
